# revision 18
# baseline (speedup 1.0000x reference)
"""Multi-head attention (S=2048, B=2, D=1024, H=16) on 8 Trainium2 NeuronCores.

Sharding: 8 cores = 2 batches x 4 head-groups. Each core handles ONE batch and
FOUR heads (256 local dims): QKV projections restricted to its 256 output dims,
attention, and the row-parallel slice of the output projection. The host sums
the 4 partial outputs per batch and adds bo.

Per-core schedule is built around saturating the Scalar engine (ACT), which
must evaluate exp() on all 16.7M scores (the hard floor, ~132us at 1 elem/
cycle/lane from PSUM):
  - scores for the two heads of a group are emitted as row-tiled concurrent
    matmul pairs (K=64 at PE rows 0-63 / 64-127) -> 2x score throughput,
  - each exp call covers both heads' scores for one key block (N=1024, PSUM
    [128,1024] -> SBUF fp16),
  - context (escore @ V, with a ones column for the softmax denominator)
    accumulates serially in PSUM,
  - projections and the output projection are injected into PE slack between
    attention matmuls; ACT does nothing but exp.
"""

import math

import numpy as np

S, B, D, H = 2048, 2, 1024, 16
DK = D // H               # 64
NCORES = 8
HLOC = 4                  # heads per core
DLOC = HLOC * DK          # 256 local dims
KT = D // 128             # 8 contraction tiles
NQC = S // 512            # 4 query chunks
NKB = S // 128            # 16 key blocks
NTT = S // 128            # 16 token tiles
SCALE = 1.0 / math.sqrt(DK)

_prog_cache = {}


def _build(masked: bool):
    import concourse.mybir as mybir
    import concourse.tile as tile
    from concourse import bacc

    f16 = mybir.dt.float16
    f32 = mybir.dt.float32
    EXP = mybir.ActivationFunctionType.Exp
    MUL = mybir.AluOpType.mult
    ADD = mybir.AluOpType.add

    nc = bacc.Bacc("TRN2", target_bir_lowering=False, debug=False)

    def din(name, shape, dt=f16):
        return nc.dram_tensor(name, shape, dt, kind="ExternalInput").ap()

    xq = din("xq", [D, S])             # query[:, b, :].T for this core's batch
    xk = din("xk", [D, S])
    xv = din("xv", [D, S])
    wq = din("wq", [128, KT * DLOC])   # (p, kt, m) = Wq[hs+m, kt*128+p]
    wk = din("wk", [128, KT * DLOC])
    wv = din("wv", [128, KT * DLOC])
    wo = din("wo", [128, 2 * D])       # (p, j, n) = Wo[n, hs + j*128 + p]
    bq = din("bq", [128, 2], f32)      # (p, j) = bq[hs + j*128 + p]
    bk = din("bk", [128, 2], f32)
    bv = din("bv", [1, DLOC], f32)
    mb = din("mb", [S], f32)           # additive mask bias per key (0 / -1e30)
    out = nc.dram_tensor("out", [S, D], f16, kind="ExternalOutput").ap()
    import os
    _dbg = bool(os.environ.get("KDBG"))
    if _dbg:
        dbg_qT = nc.dram_tensor("dbg_qT", [128, 2, S], f16, kind="ExternalOutput").ap()
        dbg_kT = nc.dram_tensor("dbg_kT", [128, 2, S], f16, kind="ExternalOutput").ap()
        dbg_vv = nc.dram_tensor("dbg_vv", [128, HLOC, NKB, 68], f16, kind="ExternalOutput").ap()
        dbg_cn = nc.dram_tensor("dbg_cn", [128, 2, S], f16, kind="ExternalOutput").ap()

    with tile.TileContext(nc) as tc:
        with (
            tc.tile_pool(name="wsb", bufs=1) as wsb,
            tc.tile_pool(name="xsb", bufs=1) as xsb,
            tc.tile_pool(name="qkv", bufs=1) as qkv,
            tc.tile_pool(name="esb", bufs=16) as esb,
            tc.tile_pool(name="nrm", bufs=2) as nrm,
            tc.tile_pool(name="psc", bufs=2, space="PSUM") as psc,
            tc.tile_pool(name="pcx", bufs=1, space="PSUM") as pcx,
            tc.tile_pool(name="pj", bufs=2, space="PSUM") as pj,
        ):
            # ---- weights / constants --------------------------------------
            w_sb = {}
            for name, ap in (("wq", wq), ("wk", wk), ("wv", wv)):
                t = wsb.tile([128, KT, DLOC], f16, tag=name)
                nc.sync.dma_start(out=t, in_=ap.rearrange("p (kt m) -> p kt m", kt=KT))
                w_sb[name] = t
            wo_sb = wsb.tile([128, 2, D], f16, tag="wo")
            nc.sync.dma_start(out=wo_sb, in_=wo.rearrange("p (j n) -> p j n", j=2))
            bq_sb = wsb.tile([128, 2], f32, tag="bq")
            nc.sync.dma_start(out=bq_sb, in_=bq)
            bk_sb = wsb.tile([128, 2], f32, tag="bk")
            nc.sync.dma_start(out=bk_sb, in_=bk)
            bv_row = wsb.tile([1, DLOC], f32, tag="bv_row")
            nc.sync.dma_start(out=bv_row, in_=bv)
            bv_bc = wsb.tile([128, DLOC], f32, tag="bv_bc")
            nc.gpsimd.partition_broadcast(bv_bc, bv_row)
            mb_sb = wsb.tile([128, NKB], f32, tag="mb")
            nc.sync.dma_start(out=mb_sb, in_=mb.rearrange("(kb p) -> p kb", p=128))

            # early tiny exp so the ACT table set loads during the prefix
            twarm = wsb.tile([128, 1], f16, tag="twarm")
            nc.scalar.activation(twarm, bq_sb[:, 0:1], EXP, scale=0.0)

            # ---- persistent activations -----------------------------------
            # qT/kT: (p, j, s) = proj dim j*128+p, token s.  head h lives at
            # (j = h//2, partitions (h%2)*64 ..).  group g = heads 2g, 2g+1
            # -> slice [:, g, :] with head parity e on partition halves.
            qT = qkv.tile([128, 2, S], f16, tag="qT", name="qT")
            kT = qkv.tile([128, 2, S], f16, tag="kT", name="kT")
            vv = qkv.tile([128, HLOC, NKB, 68], f16, tag="vv", name="vv")
            nc.vector.memset(vv, 0.0)
            nc.vector.memset(vv[:, :, :, 64:65], 1.0)
            ctxn = qkv.tile([128, 2, S], f16, tag="ctxn", name="ctxn")

            # x tiles: 8 kt-tiles per tensor, loaded in 4 column chunks
            xt = {}
            for nm in ("q", "k", "v"):
                xt[nm] = [xsb.tile([128, S], f16, tag=f"x{nm}{kt}",
                                   name=f"x{nm}{kt}") for kt in range(KT)]

            def load_x_cc(nm, ap, cc, eng):
                sl = slice(cc * 512, (cc + 1) * 512)
                for kt in range(KT):
                    eng.dma_start(out=xt[nm][kt][:, sl],
                                  in_=ap[kt * 128:(kt + 1) * 128, sl])

            # ---- emission helpers -----------------------------------------
            def proj_qk_half(which, qc, j):
                """One accumulation chain: 8 matmuls + bias add -> qT/kT."""
                w, bias, dst = (("wq", bq_sb, qT) if which == "q"
                                else ("wk", bk_sb, kT))
                x = xt[which]
                ps = pj.tile([128, 512], f32, tag="pj", name=f"p{which}{qc}{j}")
                sl = slice(qc * 512, (qc + 1) * 512)
                for kt in range(KT):
                    nc.tensor.matmul(ps, w_sb[w][:, kt, j * 128:(j + 1) * 128],
                                     x[kt][:, sl],
                                     start=(kt == 0), stop=(kt == KT - 1))
                nc.vector.tensor_scalar(out=dst[:, j, sl], in0=ps,
                                        scalar1=bias[:, j:j + 1], scalar2=None,
                                        op0=ADD)

            def proj_v_tt(tt):
                ps = pj.tile([128, 512], f32, tag="pj", name=f"pv{tt}")
                tsl = slice(tt * 128, (tt + 1) * 128)
                for kt in range(KT):
                    nc.tensor.matmul(ps[:, 0:DLOC], xt["v"][kt][:, tsl],
                                     w_sb["wv"][:, kt, :],
                                     start=(kt == 0), stop=(kt == KT - 1))
                for h in range(HLOC):
                    nc.vector.tensor_tensor(
                        out=vv[:, h, tt, 0:64],
                        in0=ps[:, h * 64:(h + 1) * 64],
                        in1=bv_bc[:, h * 64:(h + 1) * 64], op=ADD)

            def outproj_tt(tt):
                tsl = slice(tt * 128, (tt + 1) * 128)
                for eh in range(2):
                    po = pj.tile([128, 512], f32, tag="pj", name=f"po{tt}{eh}")
                    for j in range(2):
                        nc.tensor.matmul(po, ctxn[:, j, tsl],
                                         wo_sb[:, j, eh * 512:(eh + 1) * 512],
                                         start=(j == 0), stop=(j == 1))
                    oc = esb.tile([128, 512], f16, tag="oc", name=f"oc{tt}{eh}")
                    nc.vector.tensor_copy(oc, po)
                    nc.gpsimd.dma_start(out=out[tsl, eh * 512:(eh + 1) * 512],
                                        in_=oc)

            # ---- inject scheduler -----------------------------------------
            # The tile framework orders dependencies by EMISSION order, so a
            # consumer must be emitted after its producer.  State tracks which
            # projection units have been emitted; attention pulls producers
            # out of the work queue early when it needs them, and delays ctx
            # emission until the matching v tile's projection is emitted.
            st = {"v_ready": 0, "k": [0] * NQC, "q": [0] * NQC}
            work = []

            def run_unit(u):
                kind, idx, thunk = u
                thunk()
                if kind == "v":
                    st["v_ready"] = idx + 1
                elif kind in ("k", "q"):
                    st[kind][idx] += 1

            def ensure(kind, idx):
                if kind == "v":
                    need = lambda: st["v_ready"] > idx
                else:
                    need = lambda: st[kind][idx] >= 2
                while not need():
                    i = next(i for i, u in enumerate(work)
                             if u[0] == kind and u[1] <= idx)
                    run_unit(work.pop(i))

            # ---- attention ------------------------------------------------
            def attn_qc(g, qc, n_inject):
                qsl = slice(qc * 512, (qc + 1) * 512)
                ensure("q", qc)
                pctx = [pcx.tile([65, 512], f32, tag=f"cx{e}", name=f"cx{g}{qc}{e}")
                        for e in range(2)]
                esc = {}

                def scores_exp(kb):
                    ps = psc.tile([128, 1024], f32, tag="sc", name=f"sc{g}{qc}{kb}")
                    ksl = slice(kb * 128, (kb + 1) * 128)
                    for e in range(2):
                        nc.tensor.matmul(
                            ps[:, e * 512:(e + 1) * 512],
                            kT[e * 64:(e + 1) * 64, g, ksl],
                            qT[e * 64:(e + 1) * 64, g, qsl],
                            start=True, stop=True,
                            tile_position=(e * 64, 0))
                    et = esb.tile([128, 1024], f16, tag="e", name=f"e{g}{qc}{kb}")
                    if masked:
                        nc.scalar.activation(et, ps, EXP,
                                             bias=mb_sb[:, kb:kb + 1], scale=SCALE)
                    else:
                        nc.scalar.activation(et, ps, EXP, scale=SCALE)
                    esc[kb] = et

                def ctx(kb):
                    # always called in ascending kb order starting at 0
                    for e in range(2):
                        nc.tensor.matmul(pctx[e], vv[:, 2 * g + e, kb, 0:65],
                                         esc[kb][:, e * 512:(e + 1) * 512],
                                         start=(kb == 0), stop=(kb == NKB - 1))
                    del esc[kb]

                scores_done = [0]

                def drain_ctx(lag):
                    while esc:
                        kb = min(esc)
                        if kb >= st["v_ready"] or kb > scores_done[0] - lag:
                            break
                        ctx(kb)

                for kb in range(NKB):
                    ensure("k", kb // 4)
                    scores_exp(kb)
                    scores_done[0] = kb + 1
                    if work and n_inject > 0:
                        run_unit(work.pop(0))
                        n_inject -= 1
                    drain_ctx(lag=2)
                while esc:
                    kb = min(esc)
                    if kb >= st["v_ready"]:
                        ensure("v", kb)
                    ctx(kb)

                # normalize -> ctxn
                for e in range(2):
                    cl = nrm.tile([1, 512], f32, tag="cl", name="cl")
                    nc.vector.tensor_copy(cl, pctx[e][64:65, :])
                    rl = nrm.tile([1, 512], f32, tag="rl", name="rl")
                    nc.vector.reciprocal_approx_fast(rl, cl)
                    rlb = nrm.tile([64, 512], f32, tag="rlb", name="rlb")
                    nc.gpsimd.partition_broadcast(rlb, rl)
                    nc.vector.tensor_tensor(
                        out=ctxn[e * 64:(e + 1) * 64, g, qsl],
                        in0=pctx[e][0:64, :], in1=rlb, op=MUL)

            def nothing():
                pass

            # ---- prefix ---------------------------------------------------
            load_x_cc("k", xk, 0, nc.sync)
            load_x_cc("q", xq, 0, nc.scalar)
            # PE warmup on full-K junk matmuls (keeps HAM at 2.4GHz);
            # safe pattern: full 128x128 stationary at (0,0).
            for wu in range(36):
                jp = psc.tile([128, 1024], f32, tag="sc", name=f"jp{wu}")
                nc.tensor.matmul(jp[:, 0:512], w_sb["wq"][:, wu % KT, 0:128],
                                 w_sb["wk"][:, (wu % 4) * 2:(wu % 4) * 2 + 2, :],
                                 start=True, stop=True)
            for cc in range(1, 4):
                load_x_cc("k", xk, cc, nc.sync)
            proj_qk_half("k", 0, 0)
            proj_qk_half("k", 0, 1)
            load_x_cc("q", xq, 1, nc.scalar)
            proj_qk_half("q", 0, 0)
            proj_qk_half("q", 0, 1)
            for cc in range(4):
                load_x_cc("v", xv, cc, nc.gpsimd)
            load_x_cc("q", xq, 2, nc.scalar)
            load_x_cc("q", xq, 3, nc.scalar)

            # ---- main loop -------------------------------------------------
            def vt(tt):
                return ("v", tt, lambda: proj_v_tt(tt))

            def k2(c):
                return [("k", c, lambda j=j: proj_qk_half("k", c, j))
                        for j in range(2)]

            def q2(c):
                return [("q", c, lambda j=j: proj_qk_half("q", c, j))
                        for j in range(2)]

            work += k2(1) + [vt(0), vt(1)] + k2(2) + [vt(2), vt(3)] + k2(3)
            work += [vt(t) for t in range(4, 12)]
            work += q2(1) + [vt(12), vt(13)] + q2(2) + [vt(14), vt(15)] + q2(3)

            st["k"][0] = 2
            st["q"][0] = 2

            for qc in range(NQC):
                for g in range(2):
                    attn_qc(g, qc, n_inject=15)
                for tt in range(qc * 4, qc * 4 + 4):
                    work.append(("o", tt, lambda tt=tt: outproj_tt(tt)))
            while work:
                run_unit(work.pop(0))
            if _dbg:
                nc.sync.dma_start(out=dbg_qT, in_=qT)
                nc.sync.dma_start(out=dbg_kT, in_=kT)
                nc.sync.dma_start(out=dbg_vv, in_=vv)
                nc.sync.dma_start(out=dbg_cn, in_=ctxn)

    nc.compile()
    return nc


def _get_prog(masked: bool):
    if masked not in _prog_cache:
        _prog_cache[masked] = _build(masked)
    return _prog_cache[masked]


def build_in_maps(query, key, value, mask, Wq, bq, Wk, bk, Wv, bv, Wo, bo):
    """Host-side sharding: returns (in_maps, masked)."""
    def xb(x, b):  # [S, B, D] -> [D, S] fp16 contiguous for batch b
        return np.ascontiguousarray(x[:, b, :].T.astype(np.float16))

    def warr(W, hs):  # [128, KT*DLOC]
        wt = W[hs:hs + DLOC, :].T.astype(np.float16)       # [D, DLOC]
        return np.ascontiguousarray(
            wt.reshape(KT, 128, DLOC).transpose(1, 0, 2).reshape(128, KT * DLOC))

    def woarr(hs):  # [128, 2*D]
        wt = Wo[:, hs:hs + DLOC].T.astype(np.float16)      # [DLOC, D]
        return np.ascontiguousarray(
            wt.reshape(2, 128, D).transpose(1, 0, 2).reshape(128, 2 * D))

    def barr(b_, hs):  # [128, 2]
        return np.ascontiguousarray(
            b_[hs:hs + DLOC].reshape(2, 128).T.astype(np.float32))

    masked = not bool(mask.all())
    mbv = np.where(mask.reshape(S), 0.0, -1e30).astype(np.float32)

    xqb = [xb(query, b) for b in range(B)]
    xkb = [xb(key, b) for b in range(B)]
    xvb = [xb(value, b) for b in range(B)]

    in_maps = []
    for c in range(NCORES):
        b, hg = c // 4, c % 4
        hs = hg * DLOC
        in_maps.append({
            "xq": xqb[b], "xk": xkb[b], "xv": xvb[b],
            "wq": warr(Wq, hs), "wk": warr(Wk, hs), "wv": warr(Wv, hs),
            "wo": woarr(hs),
            "bq": barr(bq, hs), "bk": barr(bk, hs),
            "bv": np.ascontiguousarray(bv[hs:hs + DLOC].reshape(1, DLOC)
                                       .astype(np.float32)),
            "mb": mbv,
        })
    return in_maps, masked


def kernel(query, key, value, mask, Wq, bq, Wk, bk, Wv, bv, Wo, bo):
    from concourse.bass_utils import run_bass_kernel_spmd

    args = [np.asarray(a) for a in
            (query, key, value, mask, Wq, bq, Wk, bk, Wv, bv, Wo, bo)]
    (query, key, value, mask, Wq, bq, Wk, bk, Wv, bv, Wo, bo) = args

    in_maps, masked = build_in_maps(query, key, value, mask, Wq, bq, Wk, bk,
                                    Wv, bv, Wo, bo)
    nc = _get_prog(masked)
    res = run_bass_kernel_spmd(nc, in_maps, core_ids=list(range(NCORES)))

    outf = np.zeros((S, B, D), np.float64)
    for c in range(NCORES):
        outf[:, c // 4, :] += res.results[c]["out"]
    outf += bo.astype(np.float64)
    return outf.astype(np.float32)


# revision 26
# speedup vs baseline: 1.0466x; 1.0466x over previous
"""Multi-head attention (S=2048, B=2, D=1024, H=16) on 8 Trainium2 NeuronCores.

Sharding: 8 cores = 2 batches x 4 head-groups. Each core handles ONE batch and
FOUR heads (256 local dims): QKV projections restricted to its 256 output dims,
attention, and the row-parallel slice of the output projection. The host sums
the 4 partial outputs per batch and adds bo.

Per-core schedule is built around saturating the Scalar engine (ACT), which
must evaluate exp() on all 16.7M scores (the hard floor, ~132us at 1 elem/
cycle/lane from PSUM):
  - scores for the two heads of a group are emitted as row-tiled concurrent
    matmul pairs (K=64 at PE rows 0-63 / 64-127) -> 2x score throughput,
  - each exp call covers both heads' scores for one key block (N=1024, PSUM
    [128,1024] -> SBUF fp16),
  - context (escore @ V, with a ones column for the softmax denominator)
    accumulates serially in PSUM,
  - projections and the output projection are injected into PE slack between
    attention matmuls; ACT does nothing but exp.
"""

import math

import numpy as np

S, B, D, H = 2048, 2, 1024, 16
DK = D // H               # 64
NCORES = 8
HLOC = 4                  # heads per core
DLOC = HLOC * DK          # 256 local dims
KT = D // 128             # 8 contraction tiles
NQC = S // 512            # 4 query chunks
NKB = S // 128            # 16 key blocks
NTT = S // 128            # 16 token tiles
SCALE = 1.0 / math.sqrt(DK)

_prog_cache = {}


def _build(masked: bool):
    import concourse.mybir as mybir
    import concourse.tile as tile
    from concourse import bacc

    f16 = mybir.dt.float16
    f32 = mybir.dt.float32
    EXP = mybir.ActivationFunctionType.Exp
    MUL = mybir.AluOpType.mult
    ADD = mybir.AluOpType.add

    nc = bacc.Bacc("TRN2", target_bir_lowering=False, debug=False)

    def din(name, shape, dt=f16):
        return nc.dram_tensor(name, shape, dt, kind="ExternalInput").ap()

    xq = din("xq", [D, S])             # query[:, b, :].T for this core's batch
    xk = din("xk", [D, S])
    xv = din("xv", [D, S])
    wq = din("wq", [128, KT * DLOC])   # (p, kt, m) = Wq[hs+m, kt*128+p]
    wk = din("wk", [128, KT * DLOC])
    wv = din("wv", [128, KT * DLOC])
    wo = din("wo", [128, 2 * D])       # (p, j, n) = Wo[n, hs + j*128 + p]
    bq = din("bq", [128, 2], f32)      # (p, j) = bq[hs + j*128 + p]
    bk = din("bk", [128, 2], f32)
    bv = din("bv", [1, DLOC], f32)
    mb = din("mb", [S], f32)           # additive mask bias per key (0 / -1e30)
    out = nc.dram_tensor("out", [S, D], f16, kind="ExternalOutput").ap()
    import os
    _dbg = bool(os.environ.get("KDBG"))
    if _dbg:
        dbg_qT = nc.dram_tensor("dbg_qT", [128, 2, S], f16, kind="ExternalOutput").ap()
        dbg_kT = nc.dram_tensor("dbg_kT", [128, 2, S], f16, kind="ExternalOutput").ap()
        dbg_vv = nc.dram_tensor("dbg_vv", [128, HLOC, NKB, 68], f16, kind="ExternalOutput").ap()
        dbg_cn = nc.dram_tensor("dbg_cn", [128, 2, S], f16, kind="ExternalOutput").ap()

    with tile.TileContext(nc) as tc:
        with (
            tc.tile_pool(name="wsb", bufs=1) as wsb,
            tc.tile_pool(name="xsb", bufs=1) as xsb,
            tc.tile_pool(name="qkv", bufs=1) as qkv,
            tc.tile_pool(name="esb", bufs=16) as esb,
            tc.tile_pool(name="nrm", bufs=2) as nrm,
            tc.tile_pool(name="psc", bufs=2, space="PSUM") as psc,
            tc.tile_pool(name="pcx", bufs=1, space="PSUM") as pcx,
            tc.tile_pool(name="pj", bufs=2, space="PSUM") as pj,
        ):
            # ---- weights / constants --------------------------------------
            # DMA issue cost is ~650ns of the ISSUING engine's queue time, so
            # issues are spread: sync = wq + bv + xk + rest of weights,
            # vector = xk/xq second halves, gpsimd = xq first half + xv.
            # Scalar issues nothing (its queue must stay clear for exp).
            w_sb = {}
            for name, ap in (("wq", wq), ("wk", wk), ("wv", wv)):
                w_sb[name] = wsb.tile([128, KT, DLOC], f16, tag=name, name=name)
            wo_sb = wsb.tile([128, 2, D], f16, tag="wo")
            bq_sb = wsb.tile([128, 2], f32, tag="bq")
            bk_sb = wsb.tile([128, 2], f32, tag="bk")
            bv_row = wsb.tile([1, DLOC], f32, tag="bv_row")
            bv_bc = wsb.tile([128, DLOC], f32, tag="bv_bc")
            mb_sb = wsb.tile([128, NKB], f32, tag="mb")

            def dma_weights_early():
                nc.sync.dma_start(out=w_sb["wq"],
                                  in_=wq.rearrange("p (kt m) -> p kt m", kt=KT))
                nc.sync.dma_start(out=bv_row, in_=bv)

            def dma_weights_rest():
                nc.sync.dma_start(out=w_sb["wk"],
                                  in_=wk.rearrange("p (kt m) -> p kt m", kt=KT))
                nc.sync.dma_start(out=w_sb["wv"],
                                  in_=wv.rearrange("p (kt m) -> p kt m", kt=KT))
                nc.sync.dma_start(out=wo_sb,
                                  in_=wo.rearrange("p (j n) -> p j n", j=2))
                nc.sync.dma_start(out=bq_sb, in_=bq)
                nc.sync.dma_start(out=bk_sb, in_=bk)
                if masked:
                    nc.sync.dma_start(out=mb_sb,
                                      in_=mb.rearrange("(kb p) -> p kb", p=128))

            # early tiny exp so the ACT table set loads during the prefix
            # (input is a memset tile: no DMA dependency)
            tw0 = wsb.tile([128, 1], f32, tag="tw0")
            nc.vector.memset(tw0, 0.0)
            twarm = wsb.tile([128, 1], f16, tag="twarm")
            nc.scalar.activation(twarm, tw0, EXP, scale=0.0)

            # ---- persistent activations -----------------------------------
            # qT/kT: (p, j, s) = proj dim j*128+p, token s.  head h lives at
            # (j = h//2, partitions (h%2)*64 ..).  group g = heads 2g, 2g+1
            # -> slice [:, g, :] with head parity e on partition halves.
            qT = qkv.tile([128, 2, S], f16, tag="qT", name="qT")
            kT = qkv.tile([128, 2, S], f16, tag="kT", name="kT")
            vv = qkv.tile([128, HLOC, NKB, 68], f16, tag="vv", name="vv")
            nc.vector.memset(vv, 0.0)
            nc.vector.memset(vv[:, :, :, 64:65], 1.0)
            ctxn = qkv.tile([128, 2, S], f16, tag="ctxn", name="ctxn")

            # x tiles: 8 kt-tiles per tensor, loaded in 4 column chunks
            xt = {}
            for nm in ("q", "k", "v"):
                xt[nm] = [xsb.tile([128, S], f16, tag=f"x{nm}{kt}",
                                   name=f"x{nm}{kt}") for kt in range(KT)]

            def load_x_half(nm, ap, hh, eng):
                sl = slice(hh * 1024, (hh + 1) * 1024)
                for kt in range(KT):
                    eng.dma_start(out=xt[nm][kt][:, sl],
                                  in_=ap[kt * 128:(kt + 1) * 128, sl])

            # ---- emission helpers -----------------------------------------
            def proj_qk_half(which, qc, j):
                """One accumulation chain: 8 matmuls + bias add -> qT/kT."""
                w, bias, dst = (("wq", bq_sb, qT) if which == "q"
                                else ("wk", bk_sb, kT))
                x = xt[which]
                ps = pj.tile([128, 512], f32, tag="pj", name=f"p{which}{qc}{j}")
                sl = slice(qc * 512, (qc + 1) * 512)
                for kt in range(KT):
                    nc.tensor.matmul(ps, w_sb[w][:, kt, j * 128:(j + 1) * 128],
                                     x[kt][:, sl],
                                     start=(kt == 0), stop=(kt == KT - 1))
                nc.vector.tensor_scalar(out=dst[:, j, sl], in0=ps,
                                        scalar1=bias[:, j:j + 1], scalar2=None,
                                        op0=ADD)

            def proj_v_tt(tt):
                ps = pj.tile([128, 512], f32, tag="pj", name=f"pv{tt}")
                tsl = slice(tt * 128, (tt + 1) * 128)
                for kt in range(KT):
                    nc.tensor.matmul(ps[:, 0:DLOC], xt["v"][kt][:, tsl],
                                     w_sb["wv"][:, kt, :],
                                     start=(kt == 0), stop=(kt == KT - 1))
                for h in range(HLOC):
                    nc.vector.tensor_tensor(
                        out=vv[:, h, tt, 0:64],
                        in0=ps[:, h * 64:(h + 1) * 64],
                        in1=bv_bc[:, h * 64:(h + 1) * 64], op=ADD)

            def outproj_tt(tt):
                # ldweights reuse: each ctxn[:, j, tsl] stationary serves both
                # output halves before switching j.
                tsl = slice(tt * 128, (tt + 1) * 128)
                po = [pj.tile([128, 512], f32, tag="pj", name=f"po{tt}{eh}")
                      for eh in range(2)]
                for j in range(2):
                    for eh in range(2):
                        nc.tensor.matmul(po[eh], ctxn[:, j, tsl],
                                         wo_sb[:, j, eh * 512:(eh + 1) * 512],
                                         start=(j == 0), stop=(j == 1))
                for eh in range(2):
                    oc = esb.tile([128, 512], f16, tag="oc", name=f"oc{tt}{eh}")
                    nc.vector.tensor_copy(oc, po[eh])
                    nc.gpsimd.dma_start(out=out[tsl, eh * 512:(eh + 1) * 512],
                                        in_=oc)

            # ---- inject scheduler -----------------------------------------
            # The tile framework orders dependencies by EMISSION order, so a
            # consumer must be emitted after its producer.  State tracks which
            # projection units have been emitted; attention pulls producers
            # out of the work queue early when it needs them, and delays ctx
            # emission until the matching v tile's projection is emitted.
            st = {"v_ready": 0, "k": [0] * NQC, "q": [0] * NQC}
            work = []

            def run_unit(u):
                kind, idx, thunk = u
                thunk()
                if kind == "v":
                    st["v_ready"] = idx + 1
                elif kind in ("k", "q"):
                    st[kind][idx] += 1

            def ensure(kind, idx):
                if kind == "v":
                    need = lambda: st["v_ready"] > idx
                else:
                    need = lambda: st[kind][idx] >= 2
                while not need():
                    i = next(i for i, u in enumerate(work)
                             if u[0] == kind and u[1] <= idx)
                    run_unit(work.pop(i))

            # ---- attention ------------------------------------------------
            def attn_qc(g, qc, n_inject):
                qsl = slice(qc * 512, (qc + 1) * 512)
                ensure("q", qc)
                pctx = [pcx.tile([65, 512], f32, tag=f"cx{e}", name=f"cx{g}{qc}{e}")
                        for e in range(2)]
                esc = {}

                def scores_exp(kb):
                    ps = psc.tile([128, 1024], f32, tag="sc", name=f"sc{g}{qc}{kb}")
                    ksl = slice(kb * 128, (kb + 1) * 128)
                    for e in range(2):
                        nc.tensor.matmul(
                            ps[:, e * 512:(e + 1) * 512],
                            kT[e * 64:(e + 1) * 64, g, ksl],
                            qT[e * 64:(e + 1) * 64, g, qsl],
                            start=True, stop=True,
                            tile_position=(e * 64, 0))
                    et = esb.tile([128, 1024], f16, tag="e", name=f"e{g}{qc}{kb}")
                    if masked:
                        nc.scalar.activation(et, ps, EXP,
                                             bias=mb_sb[:, kb:kb + 1], scale=SCALE)
                    else:
                        nc.scalar.activation(et, ps, EXP, scale=SCALE)
                    esc[kb] = et

                def ctx(kb):
                    # always called in ascending kb order starting at 0
                    for e in range(2):
                        nc.tensor.matmul(pctx[e], vv[:, 2 * g + e, kb, 0:65],
                                         esc[kb][:, e * 512:(e + 1) * 512],
                                         start=(kb == 0), stop=(kb == NKB - 1))
                    del esc[kb]

                scores_done = [0]

                def drain_ctx(lag):
                    while esc:
                        kb = min(esc)
                        if kb >= st["v_ready"] or kb > scores_done[0] - lag:
                            break
                        ctx(kb)

                for kb in range(NKB):
                    ensure("k", kb // 4)
                    scores_exp(kb)
                    scores_done[0] = kb + 1
                    if work and n_inject > 0:
                        run_unit(work.pop(0))
                        n_inject -= 1
                    drain_ctx(lag=2)
                while esc:
                    kb = min(esc)
                    if kb >= st["v_ready"]:
                        ensure("v", kb)
                    ctx(kb)

                # normalize -> ctxn
                for e in range(2):
                    cl = nrm.tile([1, 512], f32, tag="cl", name="cl")
                    nc.vector.tensor_copy(cl, pctx[e][64:65, :])
                    rl = nrm.tile([1, 512], f32, tag="rl", name="rl")
                    nc.vector.reciprocal_approx_fast(rl, cl)
                    rlb = nrm.tile([64, 512], f32, tag="rlb", name="rlb")
                    nc.gpsimd.partition_broadcast(rlb, rl)
                    nc.vector.tensor_tensor(
                        out=ctxn[e * 64:(e + 1) * 64, g, qsl],
                        in0=pctx[e][0:64, :], in1=rlb, op=MUL)

            def nothing():
                pass

            # ---- prefix ---------------------------------------------------
            # DMA can only be issued from sync/scalar/gpsimd.  Scalar issues
            # only xq-h0, emitted before any exp reaches its queue.
            dma_weights_early()
            load_x_half("k", xk, 0, nc.sync)      # sync: wq, bv, xk, weights
            load_x_half("q", xq, 0, nc.scalar)    # scalar: xq-h0 (prefix only)
            nc.gpsimd.partition_broadcast(bv_bc, bv_row)
            load_x_half("v", xv, 0, nc.gpsimd)    # gpsimd: xv, then xq-h1
            dma_weights_rest()
            load_x_half("k", xk, 1, nc.sync)
            load_x_half("v", xv, 1, nc.gpsimd)
            load_x_half("q", xq, 1, nc.gpsimd)
            # PE warmup on junk matmuls (keeps HAM at 2.4GHz); full 128x128
            # stationary at (0,0) only -- never same-position K=64 pairs.
            for wu in range(14):
                jp = psc.tile([128, 1024], f32, tag="sc", name=f"jp{wu}")
                nc.tensor.matmul(jp[:, 0:512], w_sb["wq"][:, wu % KT, 0:128],
                                 w_sb["wq"][:, (wu % 4) * 2:(wu % 4) * 2 + 2, :],
                                 start=True, stop=True)
            proj_qk_half("k", 0, 0)
            proj_qk_half("k", 0, 1)
            proj_qk_half("q", 0, 0)
            proj_qk_half("q", 0, 1)

            # ---- main loop -------------------------------------------------
            def vt(tt):
                return ("v", tt, lambda: proj_v_tt(tt))

            def k2(c):
                return [("k", c, lambda j=j: proj_qk_half("k", c, j))
                        for j in range(2)]

            def q2(c):
                return [("q", c, lambda j=j: proj_qk_half("q", c, j))
                        for j in range(2)]

            work += k2(1) + [vt(0), vt(1)] + k2(2) + [vt(2), vt(3)] + k2(3)
            work += [vt(4), vt(5)] + q2(1) + [vt(6), vt(7), vt(8), vt(9)]
            work += q2(2) + [vt(10), vt(11), vt(12), vt(13)] + q2(3)
            work += [vt(14), vt(15)]

            st["k"][0] = 2
            st["q"][0] = 2

            for qc in range(NQC):
                for g in range(2):
                    attn_qc(g, qc, n_inject=15)
                for tt in range(qc * 4, qc * 4 + 4):
                    work.append(("o", tt, lambda tt=tt: outproj_tt(tt)))
            while work:
                run_unit(work.pop(0))
            if _dbg:
                nc.sync.dma_start(out=dbg_qT, in_=qT)
                nc.sync.dma_start(out=dbg_kT, in_=kT)
                nc.sync.dma_start(out=dbg_vv, in_=vv)
                nc.sync.dma_start(out=dbg_cn, in_=ctxn)

    nc.compile()
    return nc


def _get_prog(masked: bool):
    if masked not in _prog_cache:
        _prog_cache[masked] = _build(masked)
    return _prog_cache[masked]


def build_in_maps(query, key, value, mask, Wq, bq, Wk, bk, Wv, bv, Wo, bo):
    """Host-side sharding: returns (in_maps, masked)."""
    def xb(x, b):  # [S, B, D] -> [D, S] fp16 contiguous for batch b
        return np.ascontiguousarray(x[:, b, :].T.astype(np.float16))

    def warr(W, hs):  # [128, KT*DLOC]
        wt = W[hs:hs + DLOC, :].T.astype(np.float16)       # [D, DLOC]
        return np.ascontiguousarray(
            wt.reshape(KT, 128, DLOC).transpose(1, 0, 2).reshape(128, KT * DLOC))

    def woarr(hs):  # [128, 2*D]
        wt = Wo[:, hs:hs + DLOC].T.astype(np.float16)      # [DLOC, D]
        return np.ascontiguousarray(
            wt.reshape(2, 128, D).transpose(1, 0, 2).reshape(128, 2 * D))

    def barr(b_, hs):  # [128, 2]
        return np.ascontiguousarray(
            b_[hs:hs + DLOC].reshape(2, 128).T.astype(np.float32))

    masked = not bool(mask.all())
    mbv = np.where(mask.reshape(S), 0.0, -1e30).astype(np.float32)

    xqb = [xb(query, b) for b in range(B)]
    xkb = [xb(key, b) for b in range(B)]
    xvb = [xb(value, b) for b in range(B)]

    in_maps = []
    for c in range(NCORES):
        b, hg = c // 4, c % 4
        hs = hg * DLOC
        in_maps.append({
            "xq": xqb[b], "xk": xkb[b], "xv": xvb[b],
            "wq": warr(Wq, hs), "wk": warr(Wk, hs), "wv": warr(Wv, hs),
            "wo": woarr(hs),
            "bq": barr(bq, hs), "bk": barr(bk, hs),
            "bv": np.ascontiguousarray(bv[hs:hs + DLOC].reshape(1, DLOC)
                                       .astype(np.float32)),
            "mb": mbv,
        })
    return in_maps, masked


def kernel(query, key, value, mask, Wq, bq, Wk, bk, Wv, bv, Wo, bo):
    from concourse.bass_utils import run_bass_kernel_spmd

    args = [np.asarray(a) for a in
            (query, key, value, mask, Wq, bq, Wk, bk, Wv, bv, Wo, bo)]
    (query, key, value, mask, Wq, bq, Wk, bk, Wv, bv, Wo, bo) = args

    in_maps, masked = build_in_maps(query, key, value, mask, Wq, bq, Wk, bk,
                                    Wv, bv, Wo, bo)
    nc = _get_prog(masked)
    res = run_bass_kernel_spmd(nc, in_maps, core_ids=list(range(NCORES)))

    outf = np.zeros((S, B, D), np.float64)
    for c in range(NCORES):
        outf[:, c // 4, :] += res.results[c]["out"]
    outf += bo.astype(np.float64)
    return outf.astype(np.float32)
